# revision 44
# baseline (speedup 1.0000x reference)
"""Bidirectional RNN tagger on 8 trn2 NeuronCores.

Strategy (sequence-parallel windows, fused per-step tiles):
  - The tanh recurrence forgets its initial state fast (~0.45
    contraction/step with these weights); starting a window scan WARM=6
    steps early from h=0 reproduces the exact scan far below the bf16
    noise floor (~4e-3 on logits of scale 1.27). Validated on CPU:
    rel err 3.9e-3 vs the fp32 reference.
  - Core c owns sequence steps [128c, 128c+128). It scans them as
    NW=4 windows of KEEPW=32 kept steps each, ALL windows advancing in
    lockstep: scan step t processes window w's token j = 32w + t
    (j indexes the core's 134-token projection range; the bwd direction
    scans its windows in descending time, so bwd slot s holds output
    window 3-s and the host flips on unshard). This fuses the per-step
    work into [128, 256] half-tiles per direction (2 H-chunks x 4
    windows x 32 batch): 128-column matmuls, one vector add and one
    tanh per half, instead of per-[128,32]-tile ops.
  - Each m-half gets its OWN PSUM tile and accumulation group so the
    add/tanh depends on only its 8 matmuls (dep tracking is
    tile-granular, and a matmul 'start' clears the whole bank's
    accumulate-bits, so groups must be bank-sequential). k 0-1 matmuls
    go first so the next step chains off the previous step's first
    tanh half. Scan phase measures ~99% PE busy.
  - Projection xp = W_ih @ emb + bias is done once per unique token
    (windows overlap by WARM tokens; the scan reads xp with a strided
    access pattern j = t + 32w), stored bf16. Block-contiguous DRAM
    packing so every DMA is one contiguous run per partition.
  - Edge windows (core 0 fwd / core 7 bwd) pad with zero embeddings
    and a zeroed bias segment so xp==0 and h stays exactly 0 through
    the pad; projection ACT writes are split at j=WARM so the pad
    bias boundary is addressable on every core (SPMD).
  - Startup: 32 dummy matmuls warm the PE (HAM clock gate: 1.2 GHz ->
    2.4 GHz after ~3.4us of activity) during the initial DMA wait, and
    a dummy tanh preloads the ACT function table (~1.3us).
  - Classifier groups are interleaved into the scan as their kept
    steps complete; output is [128 (w,b), 128 (dir,g,ki,c)] fp32,
    host does the final permute/add.
  - bf16 operands / fp32 PSUM accumulation throughout.
  Measured (NTFF, this methodology): 161.4us full-clock (~190us in the
  throttled P0 power state) vs 524us for the previous
  per-[128,32]-tile baseline (which graded at 342584 ns).
"""

import numpy as np
import ml_dtypes

import concourse.bass as bass
import concourse.mybir as mybir
from concourse.tile import TileContext
from concourse.bass_utils import run_bass_kernel_spmd

# ---------------------------------------------------------------------------
# Workaround for walrus CoreV3 "Too many sync wait commands" on the
# TileContext kernel-tail Drain: put the global-clock waits on individual
# sync-engine NOPs (one proc each) before an unadorned drain.
import concourse.tile as _tile_mod
from concourse.vector_clock import ScopedClock, VectorClock


def _drain_and_barrier(self, tick_clock, wait_clock):
    nc = self.nc
    gc = tick_clock.global_clock
    n = len(gc)
    for p in range(n):
        if gc[p] > 0:
            vec = [0] * n
            vec[p] = gc[p]
            nop_inst = nc.sync.nop()
            wait_clock.add_sem_waits(nop_inst.ins, ScopedClock({None: VectorClock(vec)}))
    nc.sync.drain()
    nc.all_engine_barrier()
    assert self.sems is not None
    popped = nc._tile_sem_poison_stack.pop()
    assert popped is self._sem_poison
    nc.clear_and_free_semaphores(list(self.sems.allocated().values()))
    nc.all_engine_barrier()


_tile_mod.TileContext._drain_and_barrier = _drain_and_barrier

# This walrus build accepts at most ONE sync-wait command per instruction
# ("Too many sync wait commands" from CoreV2/V3 setupSyncWait otherwise).
# Split multi-wait instructions in the serialized BIR: hoist all but one
# wait onto same-engine NoOps inserted immediately before the instruction
# (identical semantics: the engine blocks at the same stream position).
import json as _json
import concourse.bass_utils as _bass_utils
import concourse.bass2jax as _bass2jax

_orig_compile_bir_kernel = _bass_utils.compile_bir_kernel


def _split_multiwaits(bir_json: bytes) -> bytes:
    d = _json.loads(bir_json)
    ctr = 0
    changed = False
    for f in d.get("functions", []):
        for blk in f.get("blocks", []):
            out = []
            for inst in blk.get("instructions", []):
                si = inst.get("sync_info")
                w = (si or {}).get("on_wait") or []
                if len(w) > 1:
                    changed = True
                    for extra in w[:-1]:
                        ctr += 1
                        out.append({
                            "debug": 0, "engine": inst["engine"], "ins": [],
                            "name": f"I-wsplit-{ctr}", "opcode": "NoOp", "outs": [],
                            "sync_info": {"on_update": [], "on_wait": [extra]},
                        })
                    si["on_wait"] = [w[-1]]
                out.append(inst)
            blk["instructions"] = out
    if not changed:
        return bir_json
    return _json.dumps(d).encode()


def _patched_compile_bir_kernel(bir_json, tmpdir, neff_name="file.neff"):
    if isinstance(bir_json, str):
        bir_json = bir_json.encode()
    return _orig_compile_bir_kernel(_split_multiwaits(bir_json), tmpdir, neff_name)


_bass_utils.compile_bir_kernel = _patched_compile_bir_kernel
for _m in (_bass2jax,):
    if getattr(_m, "compile_bir_kernel", None) is _orig_compile_bir_kernel:
        _m.compile_bir_kernel = _patched_compile_bir_kernel
# ---------------------------------------------------------------------------

BF16 = ml_dtypes.bfloat16
B = 32            # batch
S = 1024          # sequence length
H = 512           # hidden
E = 512           # embed
CH = 4            # number of 128-partition chunks of H/E
KEEP = 128        # kept steps per core
NW = 4            # windows per core
KEEPW = KEEP // NW  # 32 kept steps per window
WARM = 6          # warmup steps per window (contraction ~0.45/step).
                  # WARM=5 works on the seed-0 inputs (rel 6.4e-3) and is
                  # ~2us faster, but keeps only a 3x margin vs the 2e-2
                  # gate; WARM=6 (3.9e-3, 5x margin) is the safe choice.
SCAN = KEEPW + WARM  # 38 scanned steps per window
JR = KEEP + WARM  # 134 unique projection tokens (j indices) per direction
TOK = JR * B      # tokens per direction
SC = CH * NW * B  # 512 fused per-step columns (m, w, b)
NCORES = 8
F32 = mybir.dt.float32
DBF = mybir.dt.bfloat16

# Projection ACT-write segments (j_start, length): 16-token blocks split
# at j=WARM so the edge-pad bias boundary is addressable per-segment.
SEGS = [(0, WARM), (WARM, 16 - WARM)] + [(16 * n, 16) for n in range(1, 8)] \
    + [(128, JR - 128)]
NSEG = len(SEGS)  # 10
# projection matmul blocks (j_start, length)
BLKS = [(16 * n, 16) for n in range(8)] + [(128, JR - 128)]


def _build_nc(split_scan=True, interleave_cls=True, warm_pe=True):
    nc = bass.Bass()
    p = {}
    for d in ("f", "b"):
        # emb packed [128, CH*TOK], block-contiguous: for each projection
        # block, cols (k, j_local, b) — so each et DMA is one contiguous run
        p[f"embT_{d}"] = nc.declare_dram_parameter(f"embT_{d}", [128, CH * TOK], DBF, isOutput=False)
        # weights packed [128, CH*H]: row p, col k*H + h = W.T[k*128+p, h]
        p[f"wihT_{d}"] = nc.declare_dram_parameter(f"wihT_{d}", [128, CH * H], DBF, isOutput=False)
        p[f"whhT_{d}"] = nc.declare_dram_parameter(f"whhT_{d}", [128, CH * H], DBF, isOutput=False)
        # bias packed [128, CH*NSEG]: chunk m, segment s at column m*NSEG+s
        p[f"bias_{d}"] = nc.declare_dram_parameter(f"bias_{d}", [128, CH * NSEG], F32, isOutput=False)
    # W_cls packed [128, 16]: column (d*4+k)*2+c holds W_cls[c, d*512+k*128+p]
    p["wcls"] = nc.declare_dram_parameter("wcls", [128, 16], DBF, isOutput=False)
    # out rows = (slot, b) token partitions; cols = fwd (g, ki, c) then bwd
    # (g, ki, c). bwd slot s holds window 3-s scanned descending; the host
    # adds the two halves with the appropriate index flip.
    out = nc.declare_dram_parameter("out", [NW * B, 128], F32, isOutput=True)

    Ident = mybir.ActivationFunctionType.Identity
    Tanh = mybir.ActivationFunctionType.Tanh

    with TileContext(nc) as tc:
        with (
            tc.tile_pool(name="wpool", bufs=1) as wpool,
            tc.tile_pool(name="xpool", bufs=1) as xpool,
            tc.tile_pool(name="fpool", bufs=1) as fpool,
            tc.tile_pool(name="epool", bufs=2) as epool,
            tc.tile_pool(name="opool", bufs=2) as opool,
            tc.tile_pool(name="pp", bufs=2, space="PSUM") as pp,
            tc.tile_pool(name="sp", bufs=1, space="PSUM") as sp,
            tc.tile_pool(name="cp", bufs=1, space="PSUM") as cp,
        ):
            # ---- persistent weights / state (consolidated tiles: each tile
            # costs ~230ns of semaphore setup/teardown in the kernel
            # head/tail, so weights live in a few big tiles).
            # DMA order is deliberate: first what the projection's first
            # blocks need (emb block 0 + wih + bias), then scan-only
            # weights (whh, wcls) which aren't needed for ~50us.
            ets = {}
            # h0 gets one spare column for the ACT-table preload so the
            # warm-up matmuls (reading cols 0:128) don't dep on it
            h0 = wpool.tile([128, NW * B + 1], DBF, name="h0")
            nc.gpsimd.memset(h0[:], 0.0)
            # preload the ACT table set (tanh+identity) during the DMA wait
            nc.scalar.activation(h0[:, NW * B:], h0[:, NW * B:], Tanh)
            # per-DIRECTION weight tiles: a shared tile would make the fwd
            # projection tile-dep on the bwd weight DMA, which lands ~2us
            # later. The startup-critical tensors (emb block 0 + wih, fwd
            # first) are each SPLIT into two half-DMAs issued on the Sync
            # and GpSimd queues — transfers serialize per queue, so the
            # split halves the critical staging latency.
            def et_dma(d, j0, L, split=False):
                et = epool.tile([128, CH, L * B], DBF, name=f"emb{d}", tag=f"emb{d}")
                off = CH * B * j0
                src = p[f"embT_{d}"][:, off:off + CH * L * B].rearrange(
                    "p (k t) -> p k t", k=CH)
                if split:
                    nc.sync.dma_start(out=et[:, :2, :], in_=src[:, :2, :])
                    nc.gpsimd.dma_start(out=et[:, 2:, :], in_=src[:, 2:, :])
                else:
                    nc.sync.dma_start(out=et[:], in_=src)
                return et

            wih_t = {}
            whh_t = {}
            bias = {}
            for di, d in enumerate(("f", "b")):
                j0, L = BLKS[0]
                ets[d, 0] = et_dma(d, j0, L, split=True)
                wih_t[d] = wpool.tile([128, CH, H], DBF, name=f"wih_{d}")
                wsrc = p[f"wihT_{d}"][:, :].rearrange("p (k h) -> p k h", k=CH)
                nc.sync.dma_start(out=wih_t[d][:, :2, :], in_=wsrc[:, :2, :])
                nc.gpsimd.dma_start(out=wih_t[d][:, 2:, :], in_=wsrc[:, 2:, :])
                bias[d] = wpool.tile([128, CH * NSEG], F32, name=f"bias_{d}")
                nc.sync.dma_start(out=bias[d][:], in_=p[f"bias_{d}"][:, :])
            wih = {(d, k): wih_t[d][:, k, :] for d in ("f", "b") for k in range(CH)}
            xp = {}
            feats = {}
            for d in ("f", "b"):
                # xp[d]: [128, m, j, b] bf16 — shared across windows
                xp[d] = xpool.tile([128, CH, JR, B], DBF, name=f"xp_{d}")
                # feats[d]: [128, t, m, w*b] bf16 — fused per-step layout
                feats[d] = fpool.tile([128, SCAN, CH, NW * B], DBF, name=f"feats_{d}")
            for di, d in enumerate(("f", "b")):
                whh_t[d] = wpool.tile([128, CH, H], DBF, name=f"whh_{d}")
                nc.sync.dma_start(out=whh_t[d][:],
                                  in_=p[f"whhT_{d}"][:, :].rearrange("p (k h) -> p k h", k=CH))
            whh = {(d, k): whh_t[d][:, k, :] for d in ("f", "b") for k in range(CH)}
            wcls = wpool.tile([128, 16], DBF, name="wcls")
            nc.sync.dma_start(out=wcls[:], in_=p["wcls"][:, :])

            # ---- PE warm-up: HAM unthrottles after ~3.4us of sustained
            # activity; burn the initial DMA wait on dummy matmuls so the
            # projection starts at 2.4 GHz instead of 1.2 (32 mms x ~107ns
            # cold just covers the 3.4us HAM window).
            if warm_pe:
                wps = pp.tile([128, 512], F32, name="pps", tag="pps")
                for i in range(32):
                    nc.tensor.matmul(wps[:, :128], h0[:, :128], h0[:, :128],
                                     start=(i == 0), stop=(i == 31),
                                     skip_group_check=True)

            # ---- projection: xp[d][:, m, j, b] = (W_ih @ emb)[m] + bias ----
            for bi, (j0, L) in enumerate(BLKS):
                for d in ("f", "b"):
                    if (d, bi) in ets:
                        et = ets[d, bi]
                    else:
                        et = et_dma(d, j0, L)
                    for m in range(CH):
                        ps = pp.tile([128, 512], F32, name="pps", tag="pps")
                        for k in range(CH):
                            nc.tensor.matmul(ps[:, :L * B], wih[d, k][:, m * 128:(m + 1) * 128],
                                             et[:, k, :], start=(k == 0), stop=(k == CH - 1))
                        for si, (s0, sl) in enumerate(SEGS):
                            if s0 < j0 or s0 >= j0 + L:
                                continue
                            nc.scalar.activation(
                                xp[d][:, m, s0:s0 + sl, :],
                                ps[:, (s0 - j0) * B:(s0 - j0 + sl) * B].rearrange(
                                    "p (j b) -> p j b", b=B),
                                Ident,
                                bias=bias[d][:, m * NSEG + si:m * NSEG + si + 1])

            # ---- scan: all NW windows in lockstep, fused [128, 512] tiles ----
            # Per direction-step: matmuls split in two phases by rhs chunk
            # (phase A consumes feats chunks 0-1 = previous step's first
            # tanh half; phase B chunks 2-3), and the add/tanh split in two
            # column halves (m 0-1, m 2-3) so the serial chain
            # mm -> add -> tanh of the NEXT step overlaps this step's
            # remaining matmuls. Keeps the PE (the bottleneck) saturated.
            def cls_group(g):
                # classifier for kept steps 4g..4g+3 (needs feats up to
                # scan step WARM+4g+3); interleaved into the scan so its
                # PE/ACT/DMA work hides in scan slack
                for di, d in enumerate(("f", "b")):
                    ps = cp.tile([128, 8], F32, name=f"cps{d}", tag=f"cps{d}")
                    for ki in range(4):
                        tt = WARM + 4 * g + ki
                        for k in range(CH):
                            nc.tensor.matmul(ps[:, ki * 2:(ki + 1) * 2],
                                             feats[d][:, tt, k, :],
                                             wcls[:, (di * CH + k) * 2:(di * CH + k) * 2 + 2],
                                             start=(ki == 0 and k == 0),
                                             stop=(ki == 3 and k == CH - 1),
                                             skip_group_check=True)
                    o = opool.tile([128, 8], F32, name=f"o{d}", tag=f"o{d}")
                    nc.scalar.activation(o[:], ps[:], Ident)
                    nc.sync.dma_start(out=out[:, di * 64 + g * 8:di * 64 + (g + 1) * 8],
                                      in_=o[:])

            HCH = CH // 2  # 2 chunks per half
            for t in range(SCAN):
                for d in ("f", "b"):
                    if split_scan:
                        # Separate PSUM tile (and accumulation group) per
                        # m-half so the add/tanh of a half depends on only
                        # its 8 matmuls (tile-granular dep tracking), and the
                        # next step's chain releases from the FIRST tanh
                        # half. Within a half, k 0-1 matmuls go first (they
                        # consume the previous step's first tanh half).
                        for mh in range(2):
                            psh = sp.tile([128, 2 * 128], F32, name=f"sps{d}{mh}",
                                          tag=f"sps{d}{mh}")
                            n_mm = 0
                            for ks in range(2):
                                for m in range(mh * HCH, (mh + 1) * HCH):
                                    for k in range(ks * HCH, (ks + 1) * HCH):
                                        rhs = h0[:, :NW * B] if t == 0 else feats[d][:, t - 1, k, :]
                                        nc.tensor.matmul(
                                            psh[:, (m - mh * HCH) * 128:(m - mh * HCH + 1) * 128],
                                            whh[d, k][:, m * 128:(m + 1) * 128], rhs,
                                            start=(n_mm == 0), stop=(n_mm == 7),
                                            skip_group_check=True)
                                        n_mm += 1
                            nc.vector.tensor_add(
                                psh[:].rearrange("p (m w b) -> p m w b", m=HCH, w=NW),
                                psh[:].rearrange("p (m w b) -> p m w b", m=HCH, w=NW),
                                xp[d][:, mh * HCH:(mh + 1) * HCH,
                                      t:t + (NW - 1) * KEEPW + 1:KEEPW, :])
                            nc.scalar.activation(
                                feats[d][:, t, mh * HCH:(mh + 1) * HCH, :].rearrange(
                                    "p m wb -> p (m wb)"),
                                psh[:], Tanh)
                    else:
                        ps = sp.tile([128, SC], F32, name=f"sps{d}", tag=f"sps{d}")
                        for m in range(CH):
                            for k in range(CH):
                                rhs = h0[:, :NW * B] if t == 0 else feats[d][:, t - 1, k, :]
                                nc.tensor.matmul(ps[:, m * 128:(m + 1) * 128],
                                                 whh[d, k][:, m * 128:(m + 1) * 128], rhs,
                                                 start=(k == 0), stop=(k == CH - 1))
                        nc.vector.tensor_add(
                            ps[:].rearrange("p (m w b) -> p m w b", m=CH, w=NW),
                            ps[:].rearrange("p (m w b) -> p m w b", m=CH, w=NW),
                            xp[d][:, :, t:t + (NW - 1) * KEEPW + 1:KEEPW, :])
                        nc.scalar.activation(
                            feats[d][:, t, :, :].rearrange("p m wb -> p (m wb)"),
                            ps[:], Tanh)
                if interleave_cls and t >= WARM + 3 and (t - WARM - 3) % 4 == 0:
                    cls_group((t - WARM - 3) // 4)
            if not interleave_cls:
                for g in range(8):
                    cls_group(g)
    return nc


def _prep_inputs(inputs):
    """Build the 8 per-core input maps."""
    tok = np.asarray(inputs["token_ids"]).astype(np.int64)
    emb = np.asarray(inputs["embedding"], dtype=np.float32)
    embx = np.vstack([emb, np.zeros((1, E), np.float32)]).astype(BF16)  # pad row
    PAD = emb.shape[0]

    wT = {}
    for d in ("f", "b"):
        # pack W.T [E, H] -> [128, CH*H]: row p, col k*H+h = W.T[k*128+p, h]
        for nm, key in ((f"wihT_{d}", f"W_ih_{d}"), (f"whhT_{d}", f"W_hh_{d}")):
            w = np.asarray(inputs[key], np.float32).T.astype(BF16)
            wT[nm] = np.ascontiguousarray(
                w.reshape(CH, 128, H).transpose(1, 0, 2).reshape(128, CH * H))
    bias_full = {
        "f": (np.asarray(inputs["b_ih_f"], np.float32) + np.asarray(inputs["b_hh_f"], np.float32)),
        "b": (np.asarray(inputs["b_ih_b"], np.float32) + np.asarray(inputs["b_hh_b"], np.float32)),
    }
    W_cls = np.asarray(inputs["W_cls"], np.float32)  # [2, 1024]
    wcls_pack = np.zeros((128, 16), np.float32)
    for d in range(2):
        for k in range(CH):
            for c in range(2):
                wcls_pack[:, (d * CH + k) * 2 + c] = W_cls[c, d * 512 + k * 128:d * 512 + (k + 1) * 128]
    wcls_pack = wcls_pack.astype(BF16)

    in_maps = []
    for c in range(NCORES):
        m = {"wcls": wcls_pack}
        for d in ("f", "b"):
            m[f"wihT_{d}"] = wT[f"wihT_{d}"]
            m[f"whhT_{d}"] = wT[f"whhT_{d}"]
            # token j-indices for this core/direction
            if d == "f":
                s = np.arange(128 * c - WARM, 128 * c + KEEP)
            else:
                s = np.arange(128 * c + KEEP + WARM - 1, 128 * c - 1, -1)
            valid = (s >= 0) & (s < S)
            sc = np.clip(s, 0, S - 1)
            idx = np.where(valid[:, None], tok[:, sc].T, PAD)      # [JR, B]
            embT = embx[idx.reshape(-1)].T                         # [E, TOK] bf16
            # pack [128, CH*TOK] block-contiguous: per projection block,
            # cols (k, j_local, b)
            slabs = [
                embT[:, j0 * B:(j0 + L) * B].reshape(CH, 128, L * B).transpose(1, 0, 2)
                .reshape(128, CH * L * B)
                for j0, L in BLKS
            ]
            m[f"embT_{d}"] = np.ascontiguousarray(np.concatenate(slabs, axis=1))
            # per-segment bias table: zero for segments that are entirely padding
            bt = np.zeros((128, CH * NSEG), np.float32)
            for mm in range(CH):
                for si, (s0, sl) in enumerate(SEGS):
                    if valid[s0:s0 + sl].any():
                        bt[:, mm * NSEG + si] = bias_full[d][mm * 128:(mm + 1) * 128]
            m[f"bias_{d}"] = bt
        in_maps.append(m)
    return in_maps


_NC = None


def _get_nc():
    global _NC
    if _NC is None:
        _NC = _build_nc()
    return _NC


def kernel(**inputs):
    nc = _get_nc()
    in_maps = _prep_inputs(inputs)
    res = None
    last_err = None
    for _attempt in range(3):  # rare transient NRT_EXEC_UNIT_UNRECOVERABLE
        try:
            res = run_bass_kernel_spmd(nc, in_maps, core_ids=list(range(NCORES)))
            break
        except Exception as e:  # noqa: BLE001
            last_err = e
    if res is None:
        raise last_err
    bcls = np.asarray(inputs["b_cls"], np.float32)
    out = np.empty((B, S, 2), np.float32)
    for c in range(NCORES):
        # rows = (slot, b); fwd cols: local = 32*slot + 4g + ki;
        # bwd cols: local = 127 - (32*slot + 4g + ki)
        r = res.results[c]["out"].reshape(NW, B, 2, 8, 4, 2)
        lf = r[:, :, 0].transpose(1, 0, 2, 3, 4).reshape(B, KEEP, 2)
        lb = r[:, :, 1].transpose(1, 0, 2, 3, 4).reshape(B, KEEP, 2)[:, ::-1, :]
        out[:, 128 * c:128 * (c + 1), :] = lf + lb + bcls
    return out


# revision 45
# speedup vs baseline: 1.0238x; 1.0238x over previous
"""Bidirectional RNN tagger on 8 trn2 NeuronCores.

Strategy (sequence-parallel windows, fused per-step tiles):
  - The tanh recurrence forgets its initial state fast (~0.45
    contraction/step with these weights); starting a window scan WARM=6
    steps early from h=0 reproduces the exact scan far below the bf16
    noise floor (~4e-3 on logits of scale 1.27). Validated on CPU:
    rel err 3.9e-3 vs the fp32 reference.
  - Core c owns sequence steps [128c, 128c+128). It scans them as
    NW=4 windows of KEEPW=32 kept steps each, ALL windows advancing in
    lockstep: scan step t processes window w's token j = 32w + t
    (j indexes the core's 134-token projection range; the bwd direction
    scans its windows in descending time, so bwd slot s holds output
    window 3-s and the host flips on unshard). This fuses the per-step
    work into [128, 256] half-tiles per direction (2 H-chunks x 4
    windows x 32 batch): 128-column matmuls, one vector add and one
    tanh per half, instead of per-[128,32]-tile ops.
  - Each m-half gets its OWN PSUM tile and accumulation group so the
    add/tanh depends on only its 8 matmuls (dep tracking is
    tile-granular, and a matmul 'start' clears the whole bank's
    accumulate-bits, so groups must be bank-sequential). k 0-1 matmuls
    go first so the next step chains off the previous step's first
    tanh half. Scan phase measures ~99% PE busy.
  - Projection xp = W_ih @ emb + bias is done once per unique token
    (windows overlap by WARM tokens; the scan reads xp with a strided
    access pattern j = t + 32w), stored bf16. Block-contiguous DRAM
    packing so every DMA is one contiguous run per partition.
  - Edge windows (core 0 fwd / core 7 bwd) pad with zero embeddings
    and a zeroed bias segment so xp==0 and h stays exactly 0 through
    the pad; projection ACT writes are split at j=WARM so the pad
    bias boundary is addressable on every core (SPMD).
  - Startup: 32 dummy matmuls warm the PE (HAM clock gate: 1.2 GHz ->
    2.4 GHz after ~3.4us of activity) during the initial DMA wait, and
    a dummy tanh preloads the ACT function table (~1.3us).
  - Classifier groups are interleaved into the scan as their kept
    steps complete; output is [128 (w,b), 128 (dir,g,ki,c)] fp32,
    host does the final permute/add.
  - bf16 operands / fp32 PSUM accumulation throughout.
  Measured (NTFF, this methodology): 161.4us full-clock (~190us in the
  throttled P0 power state) vs 524us for the previous
  per-[128,32]-tile baseline (which graded at 342584 ns).
"""

import numpy as np
import ml_dtypes

import concourse.bass as bass
import concourse.mybir as mybir
from concourse.tile import TileContext
from concourse.bass_utils import run_bass_kernel_spmd

# ---------------------------------------------------------------------------
# Workaround for walrus CoreV3 "Too many sync wait commands" on the
# TileContext kernel-tail Drain: put the global-clock waits on individual
# sync-engine NOPs (one proc each) before an unadorned drain.
import concourse.tile as _tile_mod
from concourse.vector_clock import ScopedClock, VectorClock


def _drain_and_barrier(self, tick_clock, wait_clock):
    nc = self.nc
    gc = tick_clock.global_clock
    n = len(gc)
    for p in range(n):
        if gc[p] > 0:
            vec = [0] * n
            vec[p] = gc[p]
            nop_inst = nc.sync.nop()
            wait_clock.add_sem_waits(nop_inst.ins, ScopedClock({None: VectorClock(vec)}))
    nc.sync.drain()
    nc.all_engine_barrier()
    assert self.sems is not None
    popped = nc._tile_sem_poison_stack.pop()
    assert popped is self._sem_poison
    nc.clear_and_free_semaphores(list(self.sems.allocated().values()))
    nc.all_engine_barrier()


_tile_mod.TileContext._drain_and_barrier = _drain_and_barrier

# This walrus build accepts at most ONE sync-wait command per instruction
# ("Too many sync wait commands" from CoreV2/V3 setupSyncWait otherwise).
# Split multi-wait instructions in the serialized BIR: hoist all but one
# wait onto same-engine NoOps inserted immediately before the instruction
# (identical semantics: the engine blocks at the same stream position).
import json as _json
import concourse.bass_utils as _bass_utils
import concourse.bass2jax as _bass2jax

_orig_compile_bir_kernel = _bass_utils.compile_bir_kernel


def _split_multiwaits(bir_json: bytes) -> bytes:
    d = _json.loads(bir_json)
    ctr = 0
    changed = False
    for f in d.get("functions", []):
        for blk in f.get("blocks", []):
            out = []
            for inst in blk.get("instructions", []):
                si = inst.get("sync_info")
                w = (si or {}).get("on_wait") or []
                if len(w) > 1:
                    changed = True
                    for extra in w[:-1]:
                        ctr += 1
                        out.append({
                            "debug": 0, "engine": inst["engine"], "ins": [],
                            "name": f"I-wsplit-{ctr}", "opcode": "NoOp", "outs": [],
                            "sync_info": {"on_update": [], "on_wait": [extra]},
                        })
                    si["on_wait"] = [w[-1]]
                out.append(inst)
            blk["instructions"] = out
    if not changed:
        return bir_json
    return _json.dumps(d).encode()


def _patched_compile_bir_kernel(bir_json, tmpdir, neff_name="file.neff"):
    if isinstance(bir_json, str):
        bir_json = bir_json.encode()
    return _orig_compile_bir_kernel(_split_multiwaits(bir_json), tmpdir, neff_name)


_bass_utils.compile_bir_kernel = _patched_compile_bir_kernel
for _m in (_bass2jax,):
    if getattr(_m, "compile_bir_kernel", None) is _orig_compile_bir_kernel:
        _m.compile_bir_kernel = _patched_compile_bir_kernel
# ---------------------------------------------------------------------------

BF16 = ml_dtypes.bfloat16
B = 32            # batch
S = 1024          # sequence length
H = 512           # hidden
E = 512           # embed
CH = 4            # number of 128-partition chunks of H/E
KEEP = 128        # kept steps per core
NW = 4            # windows per core
KEEPW = KEEP // NW  # 32 kept steps per window
WARM = 5          # warmup steps per window (contraction ~0.45/step).
                  # Validated on TWO independent input draws: seed-0 rel
                  # 6.4e-3 (HW 6.65e-3), seed-1 rel 6.2e-3 — the truncation
                  # error is stable across draws (+-3%), margin >=3x vs the
                  # 2e-2 gate. (WARM=6 gives 3.9e-3 at +2us.)
SCAN = KEEPW + WARM  # 38 scanned steps per window
JR = KEEP + WARM  # 134 unique projection tokens (j indices) per direction
TOK = JR * B      # tokens per direction
SC = CH * NW * B  # 512 fused per-step columns (m, w, b)
NCORES = 8
F32 = mybir.dt.float32
DBF = mybir.dt.bfloat16

# Projection ACT-write segments (j_start, length): 16-token blocks split
# at j=WARM so the edge-pad bias boundary is addressable per-segment.
SEGS = [(0, WARM), (WARM, 16 - WARM)] + [(16 * n, 16) for n in range(1, 8)] \
    + [(128, JR - 128)]
NSEG = len(SEGS)  # 10
# projection matmul blocks (j_start, length)
BLKS = [(16 * n, 16) for n in range(8)] + [(128, JR - 128)]


def _build_nc(split_scan=True, interleave_cls=True, warm_pe=True):
    nc = bass.Bass()
    p = {}
    for d in ("f", "b"):
        # emb packed [128, CH*TOK], block-contiguous: for each projection
        # block, cols (k, j_local, b) — so each et DMA is one contiguous run
        p[f"embT_{d}"] = nc.declare_dram_parameter(f"embT_{d}", [128, CH * TOK], DBF, isOutput=False)
        # weights packed [128, CH*H]: row p, col k*H + h = W.T[k*128+p, h]
        p[f"wihT_{d}"] = nc.declare_dram_parameter(f"wihT_{d}", [128, CH * H], DBF, isOutput=False)
        p[f"whhT_{d}"] = nc.declare_dram_parameter(f"whhT_{d}", [128, CH * H], DBF, isOutput=False)
        # bias packed [128, CH*NSEG]: chunk m, segment s at column m*NSEG+s
        p[f"bias_{d}"] = nc.declare_dram_parameter(f"bias_{d}", [128, CH * NSEG], F32, isOutput=False)
    # W_cls packed [128, 16]: column (d*4+k)*2+c holds W_cls[c, d*512+k*128+p]
    p["wcls"] = nc.declare_dram_parameter("wcls", [128, 16], DBF, isOutput=False)
    # out rows = (slot, b) token partitions; cols = fwd (g, ki, c) then bwd
    # (g, ki, c). bwd slot s holds window 3-s scanned descending; the host
    # adds the two halves with the appropriate index flip.
    out = nc.declare_dram_parameter("out", [NW * B, 128], F32, isOutput=True)

    Ident = mybir.ActivationFunctionType.Identity
    Tanh = mybir.ActivationFunctionType.Tanh

    with TileContext(nc) as tc:
        with (
            tc.tile_pool(name="wpool", bufs=1) as wpool,
            tc.tile_pool(name="xpool", bufs=1) as xpool,
            tc.tile_pool(name="fpool", bufs=1) as fpool,
            tc.tile_pool(name="epool", bufs=3) as epool,
            tc.tile_pool(name="opool", bufs=2) as opool,
            tc.tile_pool(name="pp", bufs=2, space="PSUM") as pp,
            tc.tile_pool(name="sp", bufs=1, space="PSUM") as sp,
            tc.tile_pool(name="cp", bufs=1, space="PSUM") as cp,
        ):
            # ---- persistent weights / state (consolidated tiles: each tile
            # costs ~230ns of semaphore setup/teardown in the kernel
            # head/tail, so weights live in a few big tiles).
            # DMA order is deliberate: first what the projection's first
            # blocks need (emb block 0 + wih + bias), then scan-only
            # weights (whh, wcls) which aren't needed for ~50us.
            ets = {}
            # h0 gets one spare column for the ACT-table preload so the
            # warm-up matmuls (reading cols 0:128) don't dep on it
            h0 = wpool.tile([128, NW * B + 1], DBF, name="h0")
            nc.gpsimd.memset(h0[:], 0.0)
            # preload the ACT table set (tanh+identity) during the DMA wait
            nc.scalar.activation(h0[:, NW * B:], h0[:, NW * B:], Tanh)
            # per-DIRECTION weight tiles: a shared tile would make the fwd
            # projection tile-dep on the bwd weight DMA, which lands ~2us
            # later. The startup-critical tensors (emb block 0 + wih, fwd
            # first) are each SPLIT into two half-DMAs issued on the Sync
            # and GpSimd queues — transfers serialize per queue, so the
            # split halves the critical staging latency.
            def et_dma(d, j0, L, split=False):
                et = epool.tile([128, CH, L * B], DBF, name=f"emb{d}", tag=f"emb{d}")
                off = CH * B * j0
                src = p[f"embT_{d}"][:, off:off + CH * L * B].rearrange(
                    "p (k t) -> p k t", k=CH)
                if split:
                    nc.sync.dma_start(out=et[:, :2, :], in_=src[:, :2, :])
                    nc.gpsimd.dma_start(out=et[:, 2:, :], in_=src[:, 2:, :])
                else:
                    nc.sync.dma_start(out=et[:], in_=src)
                return et

            wih_t = {}
            whh_t = {}
            bias = {}
            for di, d in enumerate(("f", "b")):
                j0, L = BLKS[0]
                ets[d, 0] = et_dma(d, j0, L, split=True)
                wih_t[d] = wpool.tile([128, CH, H], DBF, name=f"wih_{d}")
                wsrc = p[f"wihT_{d}"][:, :].rearrange("p (k h) -> p k h", k=CH)
                nc.sync.dma_start(out=wih_t[d][:, :2, :], in_=wsrc[:, :2, :])
                nc.gpsimd.dma_start(out=wih_t[d][:, 2:, :], in_=wsrc[:, 2:, :])
                bias[d] = wpool.tile([128, CH * NSEG], F32, name=f"bias_{d}")
                nc.sync.dma_start(out=bias[d][:], in_=p[f"bias_{d}"][:, :])
            wih = {(d, k): wih_t[d][:, k, :] for d in ("f", "b") for k in range(CH)}
            xp = {}
            feats = {}
            for d in ("f", "b"):
                # xp[d]: [128, m, j, b] bf16 — shared across windows
                xp[d] = xpool.tile([128, CH, JR, B], DBF, name=f"xp_{d}")
                # feats[d]: [128, t, m, w*b] bf16 — fused per-step layout
                feats[d] = fpool.tile([128, SCAN, CH, NW * B], DBF, name=f"feats_{d}")
            for di, d in enumerate(("f", "b")):
                whh_t[d] = wpool.tile([128, CH, H], DBF, name=f"whh_{d}")
                nc.sync.dma_start(out=whh_t[d][:],
                                  in_=p[f"whhT_{d}"][:, :].rearrange("p (k h) -> p k h", k=CH))
            whh = {(d, k): whh_t[d][:, k, :] for d in ("f", "b") for k in range(CH)}
            wcls = wpool.tile([128, 16], DBF, name="wcls")
            nc.sync.dma_start(out=wcls[:], in_=p["wcls"][:, :])

            # ---- PE warm-up: HAM unthrottles after ~3.4us of sustained
            # activity; burn the initial DMA wait on dummy matmuls so the
            # projection starts at 2.4 GHz instead of 1.2 (32 mms x ~107ns
            # cold just covers the 3.4us HAM window).
            if warm_pe:
                wps = pp.tile([128, 512], F32, name="pps", tag="pps")
                for i in range(32):
                    nc.tensor.matmul(wps[:, :128], h0[:, :128], h0[:, :128],
                                     start=(i == 0), stop=(i == 31),
                                     skip_group_check=True)

            # ---- projection: xp[d][:, m, j, b] = (W_ih @ emb)[m] + bias ----
            for bi, (j0, L) in enumerate(BLKS):
                for d in ("f", "b"):
                    if (d, bi) in ets:
                        et = ets[d, bi]
                    else:
                        et = et_dma(d, j0, L)
                    for m in range(CH):
                        ps = pp.tile([128, 512], F32, name="pps", tag="pps")
                        for k in range(CH):
                            nc.tensor.matmul(ps[:, :L * B], wih[d, k][:, m * 128:(m + 1) * 128],
                                             et[:, k, :], start=(k == 0), stop=(k == CH - 1))
                        for si, (s0, sl) in enumerate(SEGS):
                            if s0 < j0 or s0 >= j0 + L:
                                continue
                            nc.scalar.activation(
                                xp[d][:, m, s0:s0 + sl, :],
                                ps[:, (s0 - j0) * B:(s0 - j0 + sl) * B].rearrange(
                                    "p (j b) -> p j b", b=B),
                                Ident,
                                bias=bias[d][:, m * NSEG + si:m * NSEG + si + 1])

            # ---- scan: all NW windows in lockstep, fused [128, 512] tiles ----
            # Per direction-step: matmuls split in two phases by rhs chunk
            # (phase A consumes feats chunks 0-1 = previous step's first
            # tanh half; phase B chunks 2-3), and the add/tanh split in two
            # column halves (m 0-1, m 2-3) so the serial chain
            # mm -> add -> tanh of the NEXT step overlaps this step's
            # remaining matmuls. Keeps the PE (the bottleneck) saturated.
            def cls_group(g):
                # classifier for kept steps 4g..4g+3 (needs feats up to
                # scan step WARM+4g+3); interleaved into the scan so its
                # PE/ACT/DMA work hides in scan slack
                for di, d in enumerate(("f", "b")):
                    ps = cp.tile([128, 8], F32, name=f"cps{d}", tag=f"cps{d}")
                    for ki in range(4):
                        tt = WARM + 4 * g + ki
                        for k in range(CH):
                            nc.tensor.matmul(ps[:, ki * 2:(ki + 1) * 2],
                                             feats[d][:, tt, k, :],
                                             wcls[:, (di * CH + k) * 2:(di * CH + k) * 2 + 2],
                                             start=(ki == 0 and k == 0),
                                             stop=(ki == 3 and k == CH - 1),
                                             skip_group_check=True)
                    o = opool.tile([128, 8], F32, name=f"o{d}", tag=f"o{d}")
                    nc.scalar.activation(o[:], ps[:], Ident)
                    nc.sync.dma_start(out=out[:, di * 64 + g * 8:di * 64 + (g + 1) * 8],
                                      in_=o[:])

            HCH = CH // 2  # 2 chunks per half
            for t in range(SCAN):
                for d in ("f", "b"):
                    if split_scan:
                        # Separate PSUM tile (and accumulation group) per
                        # m-half so the add/tanh of a half depends on only
                        # its 8 matmuls (tile-granular dep tracking), and the
                        # next step's chain releases from the FIRST tanh
                        # half. Within a half, k 0-1 matmuls go first (they
                        # consume the previous step's first tanh half).
                        for mh in range(2):
                            psh = sp.tile([128, 2 * 128], F32, name=f"sps{d}{mh}",
                                          tag=f"sps{d}{mh}")
                            n_mm = 0
                            for ks in range(2):
                                for m in range(mh * HCH, (mh + 1) * HCH):
                                    for k in range(ks * HCH, (ks + 1) * HCH):
                                        rhs = h0[:, :NW * B] if t == 0 else feats[d][:, t - 1, k, :]
                                        nc.tensor.matmul(
                                            psh[:, (m - mh * HCH) * 128:(m - mh * HCH + 1) * 128],
                                            whh[d, k][:, m * 128:(m + 1) * 128], rhs,
                                            start=(n_mm == 0), stop=(n_mm == 7),
                                            skip_group_check=True)
                                        n_mm += 1
                            nc.vector.tensor_add(
                                psh[:].rearrange("p (m w b) -> p m w b", m=HCH, w=NW),
                                psh[:].rearrange("p (m w b) -> p m w b", m=HCH, w=NW),
                                xp[d][:, mh * HCH:(mh + 1) * HCH,
                                      t:t + (NW - 1) * KEEPW + 1:KEEPW, :])
                            nc.scalar.activation(
                                feats[d][:, t, mh * HCH:(mh + 1) * HCH, :].rearrange(
                                    "p m wb -> p (m wb)"),
                                psh[:], Tanh)
                    else:
                        ps = sp.tile([128, SC], F32, name=f"sps{d}", tag=f"sps{d}")
                        for m in range(CH):
                            for k in range(CH):
                                rhs = h0[:, :NW * B] if t == 0 else feats[d][:, t - 1, k, :]
                                nc.tensor.matmul(ps[:, m * 128:(m + 1) * 128],
                                                 whh[d, k][:, m * 128:(m + 1) * 128], rhs,
                                                 start=(k == 0), stop=(k == CH - 1))
                        nc.vector.tensor_add(
                            ps[:].rearrange("p (m w b) -> p m w b", m=CH, w=NW),
                            ps[:].rearrange("p (m w b) -> p m w b", m=CH, w=NW),
                            xp[d][:, :, t:t + (NW - 1) * KEEPW + 1:KEEPW, :])
                        nc.scalar.activation(
                            feats[d][:, t, :, :].rearrange("p m wb -> p (m wb)"),
                            ps[:], Tanh)
                if interleave_cls and t >= WARM + 3 and (t - WARM - 3) % 4 == 0:
                    cls_group((t - WARM - 3) // 4)
            if not interleave_cls:
                for g in range(8):
                    cls_group(g)
    return nc


def _prep_inputs(inputs):
    """Build the 8 per-core input maps."""
    tok = np.asarray(inputs["token_ids"]).astype(np.int64)
    emb = np.asarray(inputs["embedding"], dtype=np.float32)
    embx = np.vstack([emb, np.zeros((1, E), np.float32)]).astype(BF16)  # pad row
    PAD = emb.shape[0]

    wT = {}
    for d in ("f", "b"):
        # pack W.T [E, H] -> [128, CH*H]: row p, col k*H+h = W.T[k*128+p, h]
        for nm, key in ((f"wihT_{d}", f"W_ih_{d}"), (f"whhT_{d}", f"W_hh_{d}")):
            w = np.asarray(inputs[key], np.float32).T.astype(BF16)
            wT[nm] = np.ascontiguousarray(
                w.reshape(CH, 128, H).transpose(1, 0, 2).reshape(128, CH * H))
    bias_full = {
        "f": (np.asarray(inputs["b_ih_f"], np.float32) + np.asarray(inputs["b_hh_f"], np.float32)),
        "b": (np.asarray(inputs["b_ih_b"], np.float32) + np.asarray(inputs["b_hh_b"], np.float32)),
    }
    W_cls = np.asarray(inputs["W_cls"], np.float32)  # [2, 1024]
    wcls_pack = np.zeros((128, 16), np.float32)
    for d in range(2):
        for k in range(CH):
            for c in range(2):
                wcls_pack[:, (d * CH + k) * 2 + c] = W_cls[c, d * 512 + k * 128:d * 512 + (k + 1) * 128]
    wcls_pack = wcls_pack.astype(BF16)

    in_maps = []
    for c in range(NCORES):
        m = {"wcls": wcls_pack}
        for d in ("f", "b"):
            m[f"wihT_{d}"] = wT[f"wihT_{d}"]
            m[f"whhT_{d}"] = wT[f"whhT_{d}"]
            # token j-indices for this core/direction
            if d == "f":
                s = np.arange(128 * c - WARM, 128 * c + KEEP)
            else:
                s = np.arange(128 * c + KEEP + WARM - 1, 128 * c - 1, -1)
            valid = (s >= 0) & (s < S)
            sc = np.clip(s, 0, S - 1)
            idx = np.where(valid[:, None], tok[:, sc].T, PAD)      # [JR, B]
            embT = embx[idx.reshape(-1)].T                         # [E, TOK] bf16
            # pack [128, CH*TOK] block-contiguous: per projection block,
            # cols (k, j_local, b)
            slabs = [
                embT[:, j0 * B:(j0 + L) * B].reshape(CH, 128, L * B).transpose(1, 0, 2)
                .reshape(128, CH * L * B)
                for j0, L in BLKS
            ]
            m[f"embT_{d}"] = np.ascontiguousarray(np.concatenate(slabs, axis=1))
            # per-segment bias table: zero for segments that are entirely padding
            bt = np.zeros((128, CH * NSEG), np.float32)
            for mm in range(CH):
                for si, (s0, sl) in enumerate(SEGS):
                    if valid[s0:s0 + sl].any():
                        bt[:, mm * NSEG + si] = bias_full[d][mm * 128:(mm + 1) * 128]
            m[f"bias_{d}"] = bt
        in_maps.append(m)
    return in_maps


_NC = None


def _get_nc():
    global _NC
    if _NC is None:
        _NC = _build_nc()
    return _NC


def kernel(**inputs):
    nc = _get_nc()
    in_maps = _prep_inputs(inputs)
    res = None
    last_err = None
    for _attempt in range(3):  # rare transient NRT_EXEC_UNIT_UNRECOVERABLE
        try:
            res = run_bass_kernel_spmd(nc, in_maps, core_ids=list(range(NCORES)))
            break
        except Exception as e:  # noqa: BLE001
            last_err = e
    if res is None:
        raise last_err
    bcls = np.asarray(inputs["b_cls"], np.float32)
    out = np.empty((B, S, 2), np.float32)
    for c in range(NCORES):
        # rows = (slot, b); fwd cols: local = 32*slot + 4g + ki;
        # bwd cols: local = 127 - (32*slot + 4g + ki)
        r = res.results[c]["out"].reshape(NW, B, 2, 8, 4, 2)
        lf = r[:, :, 0].transpose(1, 0, 2, 3, 4).reshape(B, KEEP, 2)
        lb = r[:, :, 1].transpose(1, 0, 2, 3, 4).reshape(B, KEEP, 2)[:, ::-1, :]
        out[:, 128 * c:128 * (c + 1), :] = lf + lb + bcls
    return out
